# revision 5
# baseline (speedup 1.0000x reference)
"""Pairwise squared-euclidean distance kernel for Trainium2 (8 NeuronCores).

z[i, j] = ||x_i||^2 + ||y_j||^2 - 2 * <x_i, y_j>

Sharding: x rows split across 8 cores (1024 rows each), y replicated.
Each core computes a [1024, 8192] tile of the output with no communication.

Per-core algorithm (all fp32 data, fp32r matmuls at full PE rate):
  1. Load x shard, compute xsq (row norms) and x^T via PE transpose
     (scaled by -2 during PSUM evacuation).
  2. Stream y in chunks: compute ysq per row, PE-transpose into resident
     yT [256 x 8192] (d on partitions, 2 chunks of 128).
  3. Reshape ysq from column layout [128, 64] to a single row [1, 8192]
     (PE transpose + tiny DRAM bounce).
  4. For each [128, 512] output tile, accumulate in PSUM:
        mm1: (-2 x^T)[d0, m] @ y^T[d0, n]   (start)
        mm2: (-2 x^T)[d1, m] @ y^T[d1, n]
        mm3: ones[1, m] @ ysq_row[1, n]     (stop)  -> broadcasts ysq
     then evacuate PSUM -> SBUF adding xsq as a per-partition bias
     (alternating ScalarE / VectorE), and DMA full stripes to DRAM.
"""

import os

import numpy as np

import concourse.bacc as bacc
import concourse.mybir as mybir
import concourse.tile as tile
from concourse.bass_utils import run_bass_kernel_spmd
from concourse.masks import make_identity

N_CORES = 8
N_FULL = 8192  # total x rows
M_Y = 8192  # y rows
D = 256  # feature dim
N_SHARD = N_FULL // N_CORES  # 1024 x rows per core

P = 128
NT = 512  # matmul free-dim tile (one fp32 PSUM bank)
FP32 = mybir.dt.float32
FP32R = mybir.dt.float32r
AF = mybir.ActivationFunctionType
ALU = mybir.AluOpType

_CACHE = {}
LAST_RESULTS = None


def _build():
    nc = bacc.Bacc("TRN2", target_bir_lowering=False, debug=False, num_devices=N_CORES)
    x_d = nc.dram_tensor("x", [N_SHARD, D], FP32, kind="ExternalInput").ap()
    y_d = nc.dram_tensor("y", [M_Y, D], FP32, kind="ExternalInput").ap()
    out_d = nc.dram_tensor("out", [N_SHARD, M_Y], FP32, kind="ExternalOutput").ap()

    M_TILES = N_SHARD // P  # 8 m-tiles (x rows)
    J_TILES = M_Y // P  # 64 j-tiles (y rows)
    J_CHUNKS = M_Y // 512  # 16 chunks of 512 y rows
    N_TILES = M_Y // NT  # 16 n-tiles per m-stripe

    with tile.TileContext(nc) as tc:
        with (
            tc.tile_pool(name="const", bufs=1) as const,
            tc.tile_pool(name="ystage", bufs=4) as ystage,
            tc.tile_pool(name="sq", bufs=4) as sqp,
            tc.tile_pool(name="outp", bufs=2) as outp,
            tc.tile_pool(name="dramp", bufs=1, space="DRAM") as dramp,
            tc.tile_pool(name="psmm", bufs=4, space="PSUM") as psmm,
            tc.tile_pool(name="pstp", bufs=2, space="PSUM") as pstp,
        ):
            identity = const.tile([P, P], FP32)
            make_identity(nc, identity)
            ones_f32 = const.tile([1, P], FP32)
            nc.gpsimd.memset(ones_f32[:], 1.0)
            ones_col = const.tile([1, P], FP32R)
            nc.vector.tensor_copy(ones_col[:], ones_f32[:])

            xsq = const.tile([P, M_TILES], FP32)
            ysq_col = const.tile([P, J_TILES], FP32)
            ysqT = const.tile([J_TILES, P], FP32R)
            ysq_row = const.tile([1, M_Y], FP32R)
            x_nat = const.tile([P, M_TILES, D], FP32)
            xT = [
                const.tile([P, N_SHARD], FP32R, tag=f"xT{c}", name=f"xT{c}")
                for c in range(2)
            ]
            yT = [
                const.tile([P, M_Y], FP32R, tag=f"yT{c}", name=f"yT{c}")
                for c in range(2)
            ]
            ysq_dram = dramp.tile([1, M_Y], FP32R)

            # ---- x: load, row norms, transpose (x -2 folded into evac) ----
            nc.sync.dma_start(x_nat[:], x_d.rearrange("(t p) d -> p t d", p=P))
            for t in range(M_TILES):
                sq = sqp.tile([P, D], FP32, tag="sq")
                nc.scalar.activation(
                    sq[:], x_nat[:, t, :], AF.Square, accum_out=xsq[:, t : t + 1]
                )
            for c in range(2):
                for h in range(2):
                    ps = pstp.tile([P, 512], FP32, tag="tp")
                    for s in range(4):
                        t = h * 4 + s
                        nc.tensor.transpose(
                            ps[:, s * P : (s + 1) * P],
                            x_nat[:, t, c * P : (c + 1) * P],
                            identity,
                        )
                    nc.vector.tensor_scalar_mul(
                        xT[c][:, h * 512 : (h + 1) * 512], ps[:], -2.0
                    )

            # ---- y: stream chunks of 512 rows; row norms + transposes ----
            for jc in range(J_CHUNKS):
                yst = ystage.tile([P, 4, D], FP32, tag="yst")
                nc.sync.dma_start(
                    yst[:],
                    y_d[jc * 512 : (jc + 1) * 512, :].rearrange(
                        "(t p) d -> p t d", p=P
                    ),
                )
                for s in range(4):
                    jt = jc * 4 + s
                    sq = sqp.tile([P, D], FP32, tag="sq")
                    nc.scalar.activation(
                        sq[:],
                        yst[:, s, :],
                        AF.Square,
                        accum_out=ysq_col[:, jt : jt + 1],
                    )
                for c in range(2):
                    ps = pstp.tile([P, 512], FP32, tag="tp")
                    for s in range(4):
                        nc.tensor.transpose(
                            ps[:, s * P : (s + 1) * P],
                            yst[:, s, c * P : (c + 1) * P],
                            identity,
                        )
                    nc.vector.tensor_copy(yT[c][:, jc * 512 : (jc + 1) * 512], ps[:])

            # ---- ysq: [128, 64] column layout -> [1, 8192] row layout ----
            ps = pstp.tile([P, 512], FP32, tag="tp")
            nc.tensor.transpose(ps[:J_TILES, :P], ysq_col[:], identity)
            nc.vector.tensor_copy(ysqT[:], ps[:J_TILES, :P])
            nc.sync.dma_start(
                ysq_dram[:].rearrange("o (t p) -> (o t) p", p=P), ysqT[:]
            )
            nc.sync.dma_start(ysq_row[:], ysq_dram[:])

            # ---- main: z tiles via 3-matmul PSUM accumulation ----
            for m in range(M_TILES):
                lhs0 = xT[0][:, m * P : (m + 1) * P]
                lhs1 = xT[1][:, m * P : (m + 1) * P]
                for h in range(2):
                    ot = outp.tile([P, 8 * NT], FP32, tag="ot")
                    for k in range(8):
                        n = h * 8 + k
                        pm = psmm.tile([P, NT], FP32, tag="mm")
                        nc.tensor.matmul(
                            pm[:],
                            lhs0,
                            yT[0][:, n * NT : (n + 1) * NT],
                            start=True,
                            stop=False,
                        )
                        nc.tensor.matmul(
                            pm[:],
                            lhs1,
                            yT[1][:, n * NT : (n + 1) * NT],
                            start=False,
                            stop=False,
                        )
                        nc.tensor.matmul(
                            pm[:],
                            ones_col[:],
                            ysq_row[:, n * NT : (n + 1) * NT],
                            start=False,
                            stop=True,
                        )
                        osl = ot[:, k * NT : (k + 1) * NT]
                        if n % 2 == 0:
                            nc.scalar.activation(
                                osl, pm[:], AF.Identity, bias=xsq[:, m : m + 1], scale=1.0
                            )
                        else:
                            nc.vector.tensor_scalar_add(osl, pm[:], xsq[:, m : m + 1])
                    nc.sync.dma_start(
                        out_d[m * P : (m + 1) * P, h * 8 * NT : (h + 1) * 8 * NT],
                        ot[:],
                    )

    nc.compile()
    return nc


def _get_nc():
    if "nc" not in _CACHE:
        _CACHE["nc"] = _build()
    return _CACHE["nc"]


def kernel(x: np.ndarray, y: np.ndarray) -> np.ndarray:
    global LAST_RESULTS
    x = np.ascontiguousarray(np.asarray(x, dtype=np.float32))
    y = np.ascontiguousarray(np.asarray(y, dtype=np.float32))
    assert x.shape == (N_FULL, D) and y.shape == (M_Y, D)

    nc = _get_nc()
    in_maps = [
        {"x": x[i * N_SHARD : (i + 1) * N_SHARD], "y": y} for i in range(N_CORES)
    ]
    res = run_bass_kernel_spmd(
        nc,
        in_maps,
        core_ids=list(range(N_CORES)),
        trace=bool(os.environ.get("BASS_KERNEL_TRACE")),
    )
    LAST_RESULTS = res
    return np.concatenate([res.results[i]["out"] for i in range(N_CORES)], axis=0)


# revision 6
# speedup vs baseline: 1.1774x; 1.1774x over previous
"""Pairwise squared-euclidean distance kernel for Trainium2 (8 NeuronCores).

z[i, j] = ||x_i||^2 + ||y_j||^2 - 2 * <x_i, y_j>

Sharding: x rows split across 8 cores (1024 rows each), y replicated.
Each core computes a [1024, 8192] tile of the output with no communication.

Per-core algorithm (all fp32 data, fp32r matmuls at full PE rate):
  1. Load x shard, compute xsq (row norms) and x^T via PE transpose
     (scaled by -2 during PSUM evacuation).
  2. Stream y in chunks: compute ysq per row, PE-transpose into resident
     yT [256 x 8192] (d on partitions, 2 chunks of 128).
  3. Reshape ysq from column layout [128, 64] to a single row [1, 8192]
     (PE transpose + tiny DRAM bounce).
  4. For each [128, 512] output tile, accumulate in PSUM:
        mm1: (-2 x^T)[d0, m] @ y^T[d0, n]   (start)
        mm2: (-2 x^T)[d1, m] @ y^T[d1, n]
        mm3: ones[1, m] @ ysq_row[1, n]     (stop)  -> broadcasts ysq
     then evacuate PSUM -> SBUF adding xsq as a per-partition bias
     (alternating ScalarE / VectorE), and DMA full stripes to DRAM.
"""

import os

import numpy as np

import concourse.bacc as bacc
import concourse.mybir as mybir
import concourse.tile as tile
from concourse.bass_utils import run_bass_kernel_spmd
from concourse.masks import make_identity

N_CORES = 8
N_FULL = 8192  # total x rows
M_Y = 8192  # y rows
D = 256  # feature dim
N_SHARD = N_FULL // N_CORES  # 1024 x rows per core

P = 128
NT = 512  # matmul free-dim tile (one fp32 PSUM bank)
FP32 = mybir.dt.float32
FP16 = mybir.dt.float16
AF = mybir.ActivationFunctionType
ALU = mybir.AluOpType

_CACHE = {}
LAST_RESULTS = None


def _build():
    nc = bacc.Bacc("TRN2", target_bir_lowering=False, debug=False, num_devices=N_CORES)
    x_d = nc.dram_tensor("x", [N_SHARD, D], FP32, kind="ExternalInput").ap()
    y_d = nc.dram_tensor("y", [M_Y, D], FP32, kind="ExternalInput").ap()
    out_d = nc.dram_tensor("out", [N_SHARD, M_Y], FP32, kind="ExternalOutput").ap()

    M_TILES = N_SHARD // P  # 8 m-tiles (x rows)
    J_TILES = M_Y // P  # 64 j-tiles (y rows)
    J_CHUNKS = M_Y // 512  # 16 chunks of 512 y rows
    N_TILES = M_Y // NT  # 16 n-tiles per m-stripe

    with tile.TileContext(nc) as tc:
        with (
            tc.tile_pool(name="const", bufs=1) as const,
            tc.tile_pool(name="ystage", bufs=4) as ystage,
            tc.tile_pool(name="sq", bufs=4) as sqp,
            tc.tile_pool(name="outp", bufs=2) as outp,
            tc.tile_pool(name="dramp", bufs=1, space="DRAM") as dramp,
            tc.tile_pool(name="psmm", bufs=4, space="PSUM") as psmm,
            tc.tile_pool(name="pstp", bufs=2, space="PSUM") as pstp,
        ):
            identity = const.tile([P, P], FP32)
            make_identity(nc, identity)
            ones2 = const.tile([2, P], FP16)
            nc.gpsimd.memset(ones2[:], 1.0)

            xsq = const.tile([P, M_TILES], FP32)
            ysq_col = const.tile([P, J_TILES], FP32)
            ysqT = const.tile([J_TILES, P], FP32)
            ysqT_hi = const.tile([J_TILES, P], FP16)
            ysqT_lo = const.tile([J_TILES, P], FP16)
            ysq2 = const.tile([2, M_Y], FP16)
            x_nat = const.tile([P, M_TILES, D], FP32)
            xT = [
                const.tile([P, N_SHARD], FP16, tag=f"xT{c}", name=f"xT{c}")
                for c in range(2)
            ]
            yT = [
                const.tile([P, M_Y], FP16, tag=f"yT{c}", name=f"yT{c}")
                for c in range(2)
            ]
            ysq_dram2 = dramp.tile([2, M_Y], FP16)

            # ---- x: load, row norms, transpose (x -2 folded into evac) ----
            nc.sync.dma_start(x_nat[:], x_d.rearrange("(t p) d -> p t d", p=P))
            for t in range(M_TILES):
                sq = sqp.tile([P, D], FP32, tag="sq")
                nc.scalar.activation(
                    sq[:], x_nat[:, t, :], AF.Square, accum_out=xsq[:, t : t + 1]
                )
            for c in range(2):
                for h in range(2):
                    ps = pstp.tile([P, 512], FP32, tag="tp")
                    for s in range(4):
                        t = h * 4 + s
                        nc.tensor.transpose(
                            ps[:, s * P : (s + 1) * P],
                            x_nat[:, t, c * P : (c + 1) * P],
                            identity,
                        )
                    nc.vector.tensor_scalar_mul(
                        xT[c][:, h * 512 : (h + 1) * 512], ps[:], -2.0
                    )

            # ---- y: stream chunks of 512 rows; row norms + transposes ----
            for jc in range(J_CHUNKS):
                yst = ystage.tile([P, 4, D], FP32, tag="yst")
                nc.sync.dma_start(
                    yst[:],
                    y_d[jc * 512 : (jc + 1) * 512, :].rearrange(
                        "(t p) d -> p t d", p=P
                    ),
                )
                for s in range(4):
                    jt = jc * 4 + s
                    sq = sqp.tile([P, D], FP32, tag="sq")
                    nc.scalar.activation(
                        sq[:],
                        yst[:, s, :],
                        AF.Square,
                        accum_out=ysq_col[:, jt : jt + 1],
                    )
                for c in range(2):
                    ps = pstp.tile([P, 512], FP32, tag="tp")
                    for s in range(4):
                        nc.tensor.transpose(
                            ps[:, s * P : (s + 1) * P],
                            yst[:, s, c * P : (c + 1) * P],
                            identity,
                        )
                    nc.vector.tensor_copy(yT[c][:, jc * 512 : (jc + 1) * 512], ps[:])

            # ---- ysq: [128, 64] column layout -> [1, 8192] row layout ----
            ps = pstp.tile([P, 512], FP32, tag="tp")
            nc.tensor.transpose(ps[:J_TILES, :P], ysq_col[:], identity)
            nc.vector.tensor_copy(ysqT[:], ps[:J_TILES, :P])
            nc.vector.tensor_copy(ysqT_hi[:], ysqT[:])
            nc.vector.tensor_tensor(
                ysqT_lo[:], ysqT[:], ysqT_hi[:], ALU.subtract
            )
            nc.sync.dma_start(
                ysq_dram2[0:1, :].rearrange("o (t p) -> (o t) p", p=P), ysqT_hi[:]
            )
            nc.sync.dma_start(
                ysq_dram2[1:2, :].rearrange("o (t p) -> (o t) p", p=P), ysqT_lo[:]
            )
            nc.sync.dma_start(ysq2[:], ysq_dram2[:])

            # ---- main: z tiles via 3-matmul PSUM accumulation ----
            for m in range(M_TILES):
                lhs0 = xT[0][:, m * P : (m + 1) * P]
                lhs1 = xT[1][:, m * P : (m + 1) * P]
                for h in range(2):
                    ot = outp.tile([P, 8 * NT], FP32, tag="ot")
                    for k in range(8):
                        n = h * 8 + k
                        pm = psmm.tile([P, NT], FP32, tag="mm")
                        nc.tensor.matmul(
                            pm[:],
                            lhs0,
                            yT[0][:, n * NT : (n + 1) * NT],
                            start=True,
                            stop=False,
                        )
                        nc.tensor.matmul(
                            pm[:],
                            lhs1,
                            yT[1][:, n * NT : (n + 1) * NT],
                            start=False,
                            stop=False,
                        )
                        nc.tensor.matmul(
                            pm[:],
                            ones2[:],
                            ysq2[:, n * NT : (n + 1) * NT],
                            start=False,
                            stop=True,
                        )
                        osl = ot[:, k * NT : (k + 1) * NT]
                        if n % 2 == 0:
                            nc.scalar.activation(
                                osl, pm[:], AF.Identity, bias=xsq[:, m : m + 1], scale=1.0
                            )
                        else:
                            nc.vector.tensor_scalar_add(osl, pm[:], xsq[:, m : m + 1])
                    nc.sync.dma_start(
                        out_d[m * P : (m + 1) * P, h * 8 * NT : (h + 1) * 8 * NT],
                        ot[:],
                    )

    nc.compile()
    return nc


def _get_nc():
    if "nc" not in _CACHE:
        _CACHE["nc"] = _build()
    return _CACHE["nc"]


def kernel(x: np.ndarray, y: np.ndarray) -> np.ndarray:
    global LAST_RESULTS
    x = np.ascontiguousarray(np.asarray(x, dtype=np.float32))
    y = np.ascontiguousarray(np.asarray(y, dtype=np.float32))
    assert x.shape == (N_FULL, D) and y.shape == (M_Y, D)

    nc = _get_nc()
    in_maps = [
        {"x": x[i * N_SHARD : (i + 1) * N_SHARD], "y": y} for i in range(N_CORES)
    ]
    res = run_bass_kernel_spmd(
        nc,
        in_maps,
        core_ids=list(range(N_CORES)),
        trace=bool(os.environ.get("BASS_KERNEL_TRACE")),
    )
    LAST_RESULTS = res
    return np.concatenate([res.results[i]["out"] for i in range(N_CORES)], axis=0)


# revision 7
# speedup vs baseline: 1.3288x; 1.1286x over previous
"""Pairwise squared-euclidean distance kernel for Trainium2 (8 NeuronCores).

z[i, j] = ||x_i||^2 + ||y_j||^2 - 2 * <x_i, y_j>

Sharding: x rows split across 8 cores (1024 rows each), y replicated.
Each core computes a [1024, 8192] tile of the output with no communication.

Per-core algorithm (all fp32 data, fp32r matmuls at full PE rate):
  1. Load x shard, compute xsq (row norms) and x^T via PE transpose
     (scaled by -2 during PSUM evacuation).
  2. Stream y in chunks: compute ysq per row, PE-transpose into resident
     yT [256 x 8192] (d on partitions, 2 chunks of 128).
  3. Reshape ysq from column layout [128, 64] to a single row [1, 8192]
     (PE transpose + tiny DRAM bounce).
  4. For each [128, 512] output tile, accumulate in PSUM:
        mm1: (-2 x^T)[d0, m] @ y^T[d0, n]   (start)
        mm2: (-2 x^T)[d1, m] @ y^T[d1, n]
        mm3: ones[1, m] @ ysq_row[1, n]     (stop)  -> broadcasts ysq
     then evacuate PSUM -> SBUF adding xsq as a per-partition bias
     (alternating ScalarE / VectorE), and DMA full stripes to DRAM.
"""

import os

import numpy as np

import concourse.bacc as bacc
import concourse.mybir as mybir
import concourse.tile as tile
from concourse.bass_utils import run_bass_kernel_spmd
from concourse.masks import make_identity

N_CORES = 8
N_FULL = 8192  # total x rows
M_Y = 8192  # y rows
D = 256  # feature dim
N_SHARD = N_FULL // N_CORES  # 1024 x rows per core

P = 128
NT = 512  # matmul free-dim tile (one fp32 PSUM bank)
FP32 = mybir.dt.float32
FP16 = mybir.dt.float16
AF = mybir.ActivationFunctionType
ALU = mybir.AluOpType

_CACHE = {}
LAST_RESULTS = None


def _build():
    nc = bacc.Bacc("TRN2", target_bir_lowering=False, debug=False, num_devices=N_CORES)
    x_d = nc.dram_tensor("x", [N_SHARD, D], FP32, kind="ExternalInput").ap()
    y_d = nc.dram_tensor("y", [M_Y, D], FP32, kind="ExternalInput").ap()
    out_d = nc.dram_tensor("out", [N_SHARD, M_Y], FP32, kind="ExternalOutput").ap()

    M_TILES = N_SHARD // P  # 8 m-tiles (x rows)
    J_TILES = M_Y // P  # 64 j-tiles (y rows)
    J_CHUNKS = M_Y // 512  # 16 chunks of 512 y rows
    N_TILES = M_Y // NT  # 16 n-tiles per m-stripe

    with tile.TileContext(nc) as tc:
        with (
            tc.tile_pool(name="const", bufs=1) as const,
            tc.tile_pool(name="ystage", bufs=4) as ystage,
            tc.tile_pool(name="sq", bufs=4) as sqp,
            tc.tile_pool(name="outp", bufs=2) as outp,
            tc.tile_pool(name="dramp", bufs=1, space="DRAM") as dramp,
            tc.tile_pool(name="psmm", bufs=8, space="PSUM") as psmm,
        ):
            identity = const.tile([P, P], FP32)
            make_identity(nc, identity)
            ones2 = const.tile([2, P], FP16)
            nc.gpsimd.memset(ones2[:], 1.0)

            xsq = const.tile([P, M_TILES], FP32)
            ysq_col = const.tile([P, J_TILES], FP32)
            ysqT = const.tile([J_TILES, P], FP32)
            ysqT_hi = const.tile([J_TILES, P], FP16)
            ysqT_lo = const.tile([J_TILES, P], FP16)
            ysq2 = const.tile([2, M_Y], FP16)
            x_nat = const.tile([P, M_TILES, D], FP32)
            xT = [
                const.tile([P, N_SHARD], FP16, tag=f"xT{c}", name=f"xT{c}")
                for c in range(2)
            ]
            yT = [
                const.tile([P, M_Y], FP16, tag=f"yT{c}", name=f"yT{c}")
                for c in range(2)
            ]
            ysq_dram2 = dramp.tile([2, M_Y], FP16)

            # ---- x: load, row norms, transpose (x -2 folded into evac) ----
            nc.sync.dma_start(x_nat[:], x_d.rearrange("(t p) d -> p t d", p=P))
            for t in range(M_TILES):
                sq = sqp.tile([P, D], FP32, tag="sq")
                nc.scalar.activation(
                    sq[:], x_nat[:, t, :], AF.Square, accum_out=xsq[:, t : t + 1]
                )
            for c in range(2):
                for h in range(2):
                    ps = psmm.tile([P, 512], FP32, tag="mm")
                    for s in range(4):
                        t = h * 4 + s
                        nc.tensor.transpose(
                            ps[:, s * P : (s + 1) * P],
                            x_nat[:, t, c * P : (c + 1) * P],
                            identity,
                        )
                    nc.vector.tensor_scalar_mul(
                        xT[c][:, h * 512 : (h + 1) * 512], ps[:], -2.0
                    )

            # ---- y: stream chunks of 512 rows; row norms + transposes ----
            for jc in range(J_CHUNKS):
                yst = ystage.tile([P, 4, D], FP32, tag="yst")
                nc.sync.dma_start(
                    yst[:],
                    y_d[jc * 512 : (jc + 1) * 512, :].rearrange(
                        "(t p) d -> p t d", p=P
                    ),
                )
                for s in range(4):
                    jt = jc * 4 + s
                    sq = sqp.tile([P, D], FP32, tag="sq")
                    nc.scalar.activation(
                        sq[:],
                        yst[:, s, :],
                        AF.Square,
                        accum_out=ysq_col[:, jt : jt + 1],
                    )
                for c in range(2):
                    ps = psmm.tile([P, 512], FP32, tag="mm")
                    for s in range(4):
                        nc.tensor.transpose(
                            ps[:, s * P : (s + 1) * P],
                            yst[:, s, c * P : (c + 1) * P],
                            identity,
                        )
                    nc.vector.tensor_copy(yT[c][:, jc * 512 : (jc + 1) * 512], ps[:])

            # ---- ysq: [128, 64] column layout -> [1, 8192] row layout ----
            ps = psmm.tile([P, 512], FP32, tag="mm")
            nc.tensor.transpose(ps[:J_TILES, :P], ysq_col[:], identity)
            nc.vector.tensor_copy(ysqT[:], ps[:J_TILES, :P])
            nc.vector.tensor_copy(ysqT_hi[:], ysqT[:])
            nc.vector.tensor_tensor(
                ysqT_lo[:], ysqT[:], ysqT_hi[:], ALU.subtract
            )
            nc.sync.dma_start(
                ysq_dram2[0:1, :].rearrange("o (t p) -> (o t) p", p=P), ysqT_hi[:]
            )
            nc.sync.dma_start(
                ysq_dram2[1:2, :].rearrange("o (t p) -> (o t) p", p=P), ysqT_lo[:]
            )
            nc.sync.dma_start(ysq2[:], ysq_dram2[:])

            # ---- ysq broadcast tile [128, 8192] f32 via 16 fill matmuls ----
            ysqb = const.tile([P, M_Y], FP32, tag="ysqb", name="ysqb")
            for n in range(N_TILES):
                pm = psmm.tile([P, NT], FP32, tag="mm")
                nc.tensor.matmul(
                    pm[:], ones2[:], ysq2[:, n * NT : (n + 1) * NT],
                    start=True, stop=True,
                )
                if n % 2 == 0:
                    nc.scalar.copy(ysqb[:, n * NT : (n + 1) * NT], pm[:])
                else:
                    nc.vector.tensor_copy(ysqb[:, n * NT : (n + 1) * NT], pm[:])

            # ---- main: weight-reuse groups of 8 PSUM banks ----
            GRP = 8
            for m in range(M_TILES):
                lhs0 = xT[0][:, m * P : (m + 1) * P]
                lhs1 = xT[1][:, m * P : (m + 1) * P]
                for h in range(2):
                    ot = outp.tile([P, GRP * NT], FP32, tag="ot")
                    pms = [
                        psmm.tile([P, NT], FP32, tag="mm", name=f"pm_{m}_{h}_{k}")
                        for k in range(GRP)
                    ]
                    for k in range(GRP):
                        n = h * GRP + k
                        nc.tensor.matmul(
                            pms[k][:], lhs0, yT[0][:, n * NT : (n + 1) * NT],
                            start=True, stop=False,
                        )
                    for k in range(GRP):
                        n = h * GRP + k
                        nc.tensor.matmul(
                            pms[k][:], lhs1, yT[1][:, n * NT : (n + 1) * NT],
                            start=False, stop=True,
                        )
                    for k in range(GRP):
                        osl = ot[:, k * NT : (k + 1) * NT]
                        if k % 2 == 0:
                            nc.scalar.activation(
                                osl, pms[k][:], AF.Identity,
                                bias=xsq[:, m : m + 1], scale=1.0,
                            )
                        else:
                            nc.vector.tensor_scalar_add(
                                osl, pms[k][:], xsq[:, m : m + 1]
                            )
                    # ysq post-add: DVE takes 1024 cols, GpSimd 3072
                    base = h * GRP * NT
                    nc.vector.tensor_tensor(
                        ot[:, : 2 * NT], ot[:, : 2 * NT],
                        ysqb[:, base : base + 2 * NT], ALU.add,
                    )
                    nc.gpsimd.tensor_tensor(
                        ot[:, 2 * NT :], ot[:, 2 * NT :],
                        ysqb[:, base + 2 * NT : base + GRP * NT], ALU.add,
                    )
                    nc.sync.dma_start(
                        out_d[m * P : (m + 1) * P, base : base + GRP * NT],
                        ot[:],
                    )

    nc.compile()
    return nc


def _get_nc():
    if "nc" not in _CACHE:
        _CACHE["nc"] = _build()
    return _CACHE["nc"]


def kernel(x: np.ndarray, y: np.ndarray) -> np.ndarray:
    global LAST_RESULTS
    x = np.ascontiguousarray(np.asarray(x, dtype=np.float32))
    y = np.ascontiguousarray(np.asarray(y, dtype=np.float32))
    assert x.shape == (N_FULL, D) and y.shape == (M_Y, D)

    nc = _get_nc()
    in_maps = [
        {"x": x[i * N_SHARD : (i + 1) * N_SHARD], "y": y} for i in range(N_CORES)
    ]
    res = run_bass_kernel_spmd(
        nc,
        in_maps,
        core_ids=list(range(N_CORES)),
        trace=bool(os.environ.get("BASS_KERNEL_TRACE")),
    )
    LAST_RESULTS = res
    return np.concatenate([res.results[i]["out"] for i in range(N_CORES)], axis=0)
